# revision 1
# baseline (speedup 1.0000x reference)
"""CapsuleLayer dynamic-routing kernel for Trainium2 (8 NeuronCores).

Problem: x [256,1152,8] f32, route_weights [10,1152,8,16] f32 ->
out [10,256,1,16] f32 (3 routing iterations, softmax over the 1152
route nodes).

Algebra: logits accumulate additively and each delta is priors .
outputs_t, so logits_t = priors . u_t with u_1 = O_0, u_2 = O_0 + O_1.
Priors are never materialized; each iteration computes V = W_c @ u
(PE, f32r), l = sum_i x * V (DVE mult + strided reduce), e = exp(l)
with fused per-partition sum d (ACT), y^T = x^T * e^T (GPSIMD,
broadcast over i), s^T = sum_k W[k,o] y^T[k,b] (PE, 72 accumulated
matmuls), then squash / u update with tiny per-partition ops.

Sharding: 20 units of (capsule c, batch-half of 128).  Each core gets a
batch-half and 3 c-slots (cores with only 2 real units get a dummy
all-ones weight whose output is discarded).  No cross-core comms.
x^T and W^T layouts are pre-transposed on the host.
"""

import os
import sys

for _p in ("/opt/trn_rl_repo",):
    if _p not in sys.path:
        sys.path.insert(0, _p)

import numpy as np
from contextlib import ExitStack

import concourse.bass as bass
import concourse.tile as tile
from concourse import mybir
from concourse._compat import with_exitstack
from concourse.masks import make_identity

F32 = mybir.dt.float32
F32R = mybir.dt.float32r
AF = mybir.ActivationFunctionType
OP = mybir.AluOpType
AX = mybir.AxisListType

C, B, R, I, O = 10, 256, 1152, 8, 16
K = R * I            # 9216
RB = R // 128        # 9 r-blocks
KT = K // 128        # 72 k-tiles
BH = 128             # batch-half per core
NSLOT = 3            # c-slots per core
NCORES = 8

USE_F32R_V = int(os.environ.get("CAPS_F32R_V", "0"))
USE_F32R_S = int(os.environ.get("CAPS_F32R_S", "0"))
USE_F32R_S2 = int(os.environ.get("CAPS_F32R_S2", "0"))

# core k -> (batch_half, [c0, c1, c2]) ; -1 = dummy slot
CSETS = [[0, 4, 8], [1, 5, 9], [2, 6, -1], [3, 7, -1]]


def core_assignment(k):
    return k // 4, CSETS[k % 4]


@with_exitstack
def _caps_kernel(ctx: ExitStack, tc: tile.TileContext, out_ap, xh, xT_in,
                 w_aps, wT_aps):
    nc = tc.nc

    singles = ctx.enter_context(tc.tile_pool(name="singles", bufs=1))
    tw_pool = ctx.enter_context(tc.tile_pool(name="twave", bufs=2))
    y_pool = ctx.enter_context(tc.tile_pool(name="yhalf", bufs=3))
    le_pool = ctx.enter_context(tc.tile_pool(name="le", bufs=3))
    et_pool = ctx.enter_context(tc.tile_pool(name="et", bufs=2))
    small = ctx.enter_context(tc.tile_pool(name="small", bufs=3))
    psv = ctx.enter_context(tc.tile_pool(name="psv", bufs=2, space="PSUM"))
    pst = ctx.enter_context(tc.tile_pool(name="pst", bufs=2, space="PSUM"))
    pss = ctx.enter_context(tc.tile_pool(name="pss", bufs=2, space="PSUM"))

    def mmdt(ap, fast):
        if fast:
            return ap if ap.dtype == F32R else ap.bitcast(F32R)
        return ap.bitcast(F32) if ap.dtype == F32R else ap

    ident = singles.tile([128, 128], F32)
    make_identity(nc, ident)

    # ---- x^T (host-pretransposed): [p=r_off, i, rb, b], chunked by i,
    #      interleaved with the packed per-slot weights so iteration 0's
    #      matmul chain can start as soon as the first chunks land ----
    xT = singles.tile([128, I, RB, 128], F32)
    w_all = singles.tile([128, I, RB, 48], F32)
    w_re = [w_aps[s].rearrange("(rb p) i o -> p i rb o", p=128)
            for s in range(NSLOT)]
    for i in range(I):
        nc.sync.dma_start(xT[:, i], xT_in[:, i])
        for s in range(NSLOT):
            nc.sync.dma_start(w_all[:, i, :, 16 * s:16 * s + 16], w_re[s][:, i])

    # w_cT_all: [96, 9216]; slot s at partitions 32s..32s+16, (r,i)-flat.
    # Slot 0 first so iteration 1 can start while the rest stream in.
    w_cT = singles.tile([96, K], F32)
    nc.sync.dma_start(w_cT[0:16, :], wT_aps[0])

    # ---- x natural layout [b, r, i] (contiguous per partition) ----
    x_u = singles.tile([128, R, I], F32)
    nc.sync.dma_start(x_u, xh)
    for s in range(1, NSLOT):
        nc.sync.dma_start(w_cT[32 * s:32 * s + 16, :], wT_aps[s])

    # u^T per slot lives at partitions 32s..32s+16 of one [96,128] tile
    uT = singles.tile([96, 128], F32)
    ones_pp = singles.tile([128, 1], F32)
    nc.vector.memset(ones_pp, 1.0)

    u_tiles = [None] * NSLOT

    def squash_tail(s, it, sN_src, d_ap):
        """sN_src [128,16] (psum) + per-partition denom d -> O; update
        u / uT / out."""
        sN = small.tile([128, O], F32, tag="sN")
        nc.vector.tensor_copy(sN, sN_src)

        scr = small.tile([128, O], F32, tag="scr")
        q = small.tile([128, 1], F32, tag="q")
        nc.scalar.square(scr, sN)
        nc.vector.reduce_sum(q, scr, axis=AX.X)
        if d_ap is None:
            rd = ones_pp
        else:
            rd = small.tile([128, 1], F32, tag="rd")
            nc.vector.reciprocal(rd, d_ap)
        rq = small.tile([128, 1], F32, tag="rq")
        nc.scalar.sqrt(rq, q)
        a = small.tile([128, 1], F32, tag="a")
        nc.vector.tensor_mul(a, rq, rd)
        n = small.tile([128, 1], F32, tag="n")
        nc.scalar.square(n, a)
        den = small.tile([128, 1], F32, tag="den")
        nc.scalar.add(den, n, 1.0)
        rden = small.tile([128, 1], F32, tag="rden")
        nc.vector.reciprocal(rden, den)
        t2 = small.tile([128, 1], F32, tag="t2")
        nc.vector.tensor_mul(t2, a, rd)
        gf = small.tile([128, 1], F32, tag="gf")
        nc.vector.tensor_mul(gf, t2, rden)
        Ot = small.tile([128, O], F32, tag=f"O{it}_{s}", bufs=1)
        nc.vector.tensor_scalar_mul(Ot, sN, gf)

        if it == 2:
            nc.sync.dma_start(out_ap[s], Ot)
            return
        if it == 0:
            u_tiles[s] = Ot
        else:
            u2 = small.tile([128, O], F32, tag=f"u2_{s}", bufs=1)
            nc.vector.tensor_add(u2, u_tiles[s], Ot)
            u_tiles[s] = u2
        psu = pst.tile([128, 512], F32, tag="ptr")
        nc.tensor.transpose(psu[0:16, 0:128], u_tiles[s], ident)
        ustg = small.tile([16, 128], F32, tag="ustg")
        nc.scalar.copy(ustg, psu[0:16, 0:128])
        nc.sync.dma_start(uT[32 * s:32 * s + 16, :], ustg)

    # ---- iteration 0: all 3 slots in one packed matmul chain ----
    ps48 = pss.tile([48, 128], F32, tag="pss")
    for idx in range(KT):
        i, rb = idx // RB, idx % RB
        nc.tensor.matmul(
            ps48, lhsT=mmdt(w_all[:, i, rb, :], USE_F32R_S),
            rhs=mmdt(xT[:, i, rb, :], USE_F32R_S),
            start=(idx == 0), stop=(idx == KT - 1),
        )
    sT48 = small.tile([48, 128], F32, tag="sT48")
    nc.scalar.mul(sT48, ps48, 1.0 / R)
    ps2a = pst.tile([128, 512], F32, tag="ptr")
    nc.tensor.transpose(ps2a[:, 0:48], sT48, ident[0:48, 0:48])
    for s in range(NSLOT):
        squash_tail(s, 0, ps2a[:, 16 * s:16 * s + 16], None)

    # ---- iterations 1, 2: software-pipelined across slots.
    # Stage A(s): V-matmuls + x*V + reduce + exp (DVE-heavy).
    # Stage B(s): e^T transposes, y = x^T*e^T, s-matmul, squash (PE/Pool).
    # B(s) instructions are interleaved into A(s+1)'s wave loop so the
    # per-engine static schedule overlaps the stages.
    def stage_B(it, s, e_t, d):
        fast = False
        eT = et_pool.tile([128, RB, 128], F32, tag="eT")
        eTf = eT.rearrange("p rb b -> p (rb b)")
        for g, cnt in ((0, 4), (4, 4), (8, 1)):
            ps = pst.tile([128, 512], F32, tag="ptr")
            for sub in range(cnt):
                rb = g + sub
                nc.tensor.transpose(
                    ps[:, sub * 128:(sub + 1) * 128],
                    e_t[:, rb * 128:(rb + 1) * 128], ident,
                )
            if g % 2 == 0:
                nc.scalar.copy(eTf[:, g * 128:(g + cnt) * 128],
                               ps[:, 0:cnt * 128])
            else:
                nc.vector.tensor_copy(eTf[:, g * 128:(g + cnt) * 128],
                                      ps[:, 0:cnt * 128])
            yield
        ps_s = pss.tile([16, 128], F32, tag="pss")
        for qq in range(4):
            yh = y_pool.tile([128, 2, RB, 128], F32, tag="yh")
            e_bcast = bass.AP(
                tensor=eT.tensor, offset=eT.offset,
                ap=[eT.ap[0], [0, 2], [128, RB], [1, 128]],
            )
            mul_eng = nc.gpsimd if qq % 2 == 0 else nc.vector
            mul_eng.tensor_mul(yh, xT[:, qq * 2:(qq + 1) * 2, :, :], e_bcast)
            for jj in range(18):
                ii, rb = jj // RB, jj % RB
                idx = qq * 18 + jj
                nc.tensor.matmul(
                    ps_s,
                    lhsT=mmdt(w_all[:, qq * 2 + ii, rb, 16 * s:16 * s + 16],
                              fast),
                    rhs=mmdt(yh[:, ii, rb, :], fast),
                    start=(idx == 0), stop=(idx == KT - 1),
                )
            yield
        sT_sb = small.tile([16, 128], F32, tag="sTsb")
        nc.scalar.copy(sT_sb, ps_s)
        ps2 = pst.tile([128, 512], F32, tag="ptr")
        nc.tensor.transpose(ps2[:, 0:16], sT_sb, ident[0:16, 0:16])
        squash_tail(s, it, ps2[:, 0:16], d)
        yield

    def drain(gen, n=None):
        if gen is None:
            return None
        try:
            if n is None:
                while True:
                    next(gen)
            else:
                for _ in range(n):
                    next(gen)
        except StopIteration:
            return None
        return gen

    pending = None
    for it in (1, 2):
        for s in range(NSLOT):
            l_t = le_pool.tile([128, R], F32, tag="l")
            for w9 in range(9):
                pv = psv.tile([128, 1024], F32, tag="pv")
                for cc in range(2):
                    ck = w9 * 2 + cc
                    nc.tensor.matmul(
                        pv[:, cc * 512:(cc + 1) * 512],
                        lhsT=mmdt(uT[32 * s:32 * s + 16, :], USE_F32R_V),
                        rhs=mmdt(w_cT[32 * s:32 * s + 16,
                                      ck * 512:(ck + 1) * 512], USE_F32R_V),
                        start=True, stop=True, tile_position=(32 * s, 0),
                    )
                tw = tw_pool.tile([128, 128, I], F32, tag="tw")
                nc.vector.tensor_mul(
                    tw, x_u[:, w9 * 128:(w9 + 1) * 128, :],
                    pv.rearrange("p (r i) -> p r i", i=I),
                )
                nc.vector.reduce_sum(l_t[:, w9 * 128:(w9 + 1) * 128],
                                     tw, axis=AX.X)
                pending = drain(pending, 1)
            pending = drain(pending)
            e_t = l_t
            d = small.tile([128, 1], F32, tag="d")
            nc.scalar.activation(e_t, l_t, AF.Exp, accum_out=d)
            pending = stage_B(it, s, e_t, d)
    drain(pending)


def build_program():
    from concourse import bacc
    nc = bacc.Bacc("TRN2", target_bir_lowering=False, debug=False,
                   num_devices=NCORES)
    xh = nc.declare_dram_parameter("xh", [BH, R, I], F32, isOutput=False).ap()
    xT_in = nc.declare_dram_parameter("xT", [128, I, RB, BH], F32,
                                      isOutput=False).ap()
    w_aps = [
        nc.declare_dram_parameter(f"w{s}", [R, I, O], F32, isOutput=False).ap()
        for s in range(NSLOT)
    ]
    wT_aps = [
        nc.declare_dram_parameter(f"wT{s}", [O, K], F32, isOutput=False).ap()
        for s in range(NSLOT)
    ]
    out = nc.declare_dram_parameter("out", [NSLOT, BH, O], F32, isOutput=True).ap()
    with tile.TileContext(nc) as tc:
        _caps_kernel(tc, out, xh, xT_in, w_aps, wT_aps)
    nc.compile()
    return nc


def make_in_maps(x, w):
    in_maps = []
    ones_w = np.ones([R, I, O], dtype=np.float32)
    xTs = {}
    for h in range(2):
        xh_np = x[h * BH:(h + 1) * BH]  # [128 b, 1152 r, 8 i]
        # xT[p=r_off, i, rb, b] = xh[b, rb*128+p, i]
        xTs[h] = np.ascontiguousarray(
            xh_np.reshape(BH, RB, 128, I).transpose(2, 3, 1, 0))
    wTs = {c: np.ascontiguousarray(w[c].reshape(K, O).T) for c in range(C)}
    wT_ones = np.ascontiguousarray(ones_w.reshape(K, O).T)
    for k in range(NCORES):
        h, cs = core_assignment(k)
        m = {"xh": np.ascontiguousarray(x[h * BH:(h + 1) * BH]), "xT": xTs[h]}
        for s, c in enumerate(cs):
            m[f"w{s}"] = np.ascontiguousarray(w[c]) if c >= 0 else ones_w
            m[f"wT{s}"] = wTs[c] if c >= 0 else wT_ones
        in_maps.append(m)
    return in_maps


def kernel(x: np.ndarray, route_weights: np.ndarray) -> np.ndarray:
    from concourse.bass_utils import run_bass_kernel_spmd

    x = np.ascontiguousarray(x, dtype=np.float32)
    w = np.ascontiguousarray(route_weights, dtype=np.float32)
    in_maps = make_in_maps(x, w)
    nc = build_program()
    res = run_bass_kernel_spmd(nc, in_maps, list(range(NCORES)))
    global LAST_RESULTS
    LAST_RESULTS = res

    out = np.zeros([C, B, 1, O], dtype=np.float32)
    for k in range(NCORES):
        h, cs = core_assignment(k)
        o = res.results[k]["out"]
        for s, c in enumerate(cs):
            if c >= 0:
                out[c, h * BH:(h + 1) * BH, 0, :] = o[s]
    return out


if __name__ == "__main__":
    rng = np.random.default_rng(0)
    x = rng.normal(size=(B, R, I)).astype(np.float32)
    w = rng.normal(size=(C, R, I, O)).astype(np.float32)
    out = kernel(x=x, route_weights=w)
    print(out.shape, out.dtype, np.abs(out).mean())



# revision 5
# speedup vs baseline: 1.3210x; 1.3210x over previous
"""CapsuleLayer dynamic-routing kernel for Trainium2 — single-core version.

Problem: x [256,1152,8] f32, route_weights [10,1152,8,16] f32 ->
out [10,256,1,16] f32 (3 routing iterations, softmax over the 1152
route nodes).

The 8 'cores' of this environment serialize on one physical TRN2 core,
so total device work is what matters: run everything on ONE core.

Per (capsule c, iteration t in {1,2}) pair, for both batch halves h:
  V = u_t @ W_c^T               (PE, f32r, 512-wide chunks, 1 cyc/row)
  tw = x * V (f16), l = pairwise-add tree over i (DVE f16 2x)
  e0 = exp(l) with accum d (ACT); e = e0/d via Copy-with-scale -> f16
  e^T via PE transposes (f16); y = x^T * e^T broadcast (DVE/Pool f16)
  s = sum_k W[k,o] y[k,b]       (PE f16, halves paired: 256-wide rhs)
  squash -> O_t; u_2 = O_0 + O_1.
Iteration 0 (uniform probs) is a packed f16 matmul over all capsules.
W_c^T streams from HBM in [16,1024] f32r chunks through a 6-slot ring.
"""

import os
import sys

for _p in ("/opt/trn_rl_repo",):
    if _p not in sys.path:
        sys.path.insert(0, _p)

import numpy as np
from contextlib import ExitStack

import concourse.bass as bass
import concourse.tile as tile
from concourse import mybir
from concourse._compat import with_exitstack
from concourse.masks import make_identity

F32 = mybir.dt.float32
F32R = mybir.dt.float32r
F16 = mybir.dt.float16
AF = mybir.ActivationFunctionType
OP = mybir.AluOpType
AX = mybir.AxisListType

C, B, R, I, O = 10, 256, 1152, 8, 16
K = R * I            # 9216
RB = R // 128        # 9 r-blocks
KT = K // 128        # 72 k-tiles
BH = 128             # batch-half
NCORES = 1
CHW = 1024           # w_cT chunk width (elements of K)
NCH = K // CHW       # 9 chunks per (it, c)

YPOOL = int(os.environ.get("CAPS_YPOOL", "3"))    # of 8 y-blocks on Pool
JPOOL = int(os.environ.get("CAPS_JPOOL", "3"))    # j-blocks: Pool mult
JDVE = int(os.environ.get("CAPS_JDVE", "2"))      # j-blocks: DVE psum-mult
WRING = int(os.environ.get("CAPS_WRING", "5"))    # w chunk ring slots
ESPLIT = int(os.environ.get("CAPS_ESPLIT", "1"))  # eT copies: alternate DVE


@with_exitstack
def _caps_kernel(ctx: ExitStack, tc: tile.TileContext, out_ap, xu_ap, xT_ap,
                 wall_ap, wT_ap):
    nc = tc.nc

    big = ctx.enter_context(tc.tile_pool(name="big", bufs=1))
    wct_pool = ctx.enter_context(tc.tile_pool(name="wct", bufs=WRING))
    twb_pool = ctx.enter_context(tc.tile_pool(name="twb", bufs=2))
    t1_pool = ctx.enter_context(tc.tile_pool(name="t1p", bufs=1))
    l_pool = ctx.enter_context(tc.tile_pool(name="lp", bufs=int(os.environ.get("CAPS_LRING", "3"))))
    e_pool = ctx.enter_context(tc.tile_pool(name="ep", bufs=int(os.environ.get("CAPS_ERING", "3"))))
    eT_pool = ctx.enter_context(tc.tile_pool(name="etp", bufs=int(os.environ.get("CAPS_ETRING", "3"))))
    y_pool = ctx.enter_context(tc.tile_pool(name="yp", bufs=2))
    v16_pool = ctx.enter_context(tc.tile_pool(name="v16", bufs=int(os.environ.get("CAPS_VRING", "3"))))
    sT_pool = ctx.enter_context(tc.tile_pool(name="stp", bufs=2))
    small = ctx.enter_context(tc.tile_pool(name="small", bufs=4))
    psv = ctx.enter_context(tc.tile_pool(name="psv", bufs=2, space="PSUM"))
    pse = ctx.enter_context(tc.tile_pool(name="pse", bufs=int(os.environ.get("CAPS_PSE", "2")), space="PSUM"))
    pst = ctx.enter_context(tc.tile_pool(name="pst", bufs=int(os.environ.get("CAPS_PST", "1")), space="PSUM"))
    pss = ctx.enter_context(tc.tile_pool(name="pss", bufs=1, space="PSUM"))

    identf = big.tile([128, 128], F32)
    make_identity(nc, identf)
    ident16 = big.tile([128, 128], F16)
    nc.vector.tensor_copy(ident16, identf)

    # resident inputs
    xu0 = big.tile([128, R, I], F16, tag="xu0")
    xu1 = big.tile([128, R, I], F16, tag="xu1")
    xT = big.tile([128, I, RB, 2, BH], F16)
    wall = big.tile([128, I, RB, C, O], F16)
    # sliced input loads: it0 consumes (i, rb) with i outermost, so the
    # first chain can start once the i=0 slices land; the rest stream in.
    for i in range(I):
        nc.sync.dma_start(xT[:, i], xT_ap[:, i])
        nc.sync.dma_start(wall[:, i], wall_ap[:, i])
    xus = [xu0, xu1]
    for h in range(2):
        for t in range(3):
            nc.sync.dma_start(xus[h][:, 384 * t:384 * (t + 1), :],
                              xu_ap[h, :, 384 * t:384 * (t + 1), :])

    # u / uT for all 20 (c, h) units
    u_all = big.tile([128, 2 * C, O], F32)
    uT_all = big.tile([16, 2 * C, BH], F32R)

    # ---- streamed W_c^T chunk ring with a global DMA pump ----
    chunk_seq = [(it, c, j) for it in (1, 2) for c in range(C)
                 for j in range(NCH)]
    chunk_tiles = {}
    cursor = [0]

    def pump(n):
        for _ in range(n):
            if cursor[0] >= len(chunk_seq):
                return
            key = chunk_seq[cursor[0]]
            _, c_, j_ = key
            t = wct_pool.tile([16, CHW], F32R, tag="wct")
            nc.sync.dma_start(t, wT_ap[c_, :, CHW * j_:CHW * (j_ + 1)])
            chunk_tiles[key] = t
            cursor[0] += 1

    # ---- squash (pair-batched): sv <- s (frees psum fast); q = |s|^2 via
    # Square+accum; g = sqrt(q)/(1+q) as exp(0.5 ln q - ln(1+q)) so every
    # ACT func stays in the natural_log_exp table set (no reloads). ----
    def sv_copy(ps_src):
        sv = small.tile([128, O], F32, tag="sv", bufs=4, name="sv")
        nc.scalar.copy(sv, ps_src)
        return sv

    def squash_batch(it, items):
        """items: [(idx, sv_sbuf [128,16])] * 2"""
        Q = small.tile([128, 4], F32, tag="Q")
        for k, (idx, sv) in enumerate(items):
            scr = small.tile([128, O], F32, tag="scr")
            nc.scalar.activation(scr, sv, AF.Square,
                                 accum_out=Q[:, k:k + 1])
        nc.scalar.add(Q[:, 2:4], Q[:, 0:2], 1.0)
        LN = small.tile([128, 4], F32, tag="LN")
        nc.scalar.activation(LN, Q, AF.Ln)
        t2 = small.tile([128, 2], F32, tag="t2")
        nc.vector.scalar_tensor_tensor(t2, LN[:, 0:2], 0.5, LN[:, 2:4],
                                       OP.mult, OP.subtract)
        G = small.tile([128, 2], F32, tag="G")
        nc.scalar.activation(G, t2, AF.Exp)
        for k, (idx, sv) in enumerate(items):
            Ot = small.tile([128, O], F32, tag="Ot", bufs=4, name="Ot")
            nc.gpsimd.tensor_scalar_mul(Ot, sv, G[:, k:k + 1])
            if it == 2:
                nc.sync.dma_start(out_ap[idx // 2, idx % 2], Ot)
                continue
            if it == 0:
                nc.gpsimd.tensor_copy(u_all[:, idx, :], Ot)
            else:
                nc.gpsimd.tensor_add(u_all[:, idx, :], u_all[:, idx, :], Ot)
            pu = pst.tile([128, 512], F32, tag="ptr")
            nc.tensor.matmul(pu[0:16, 0:128], lhsT=u_all[:, idx, :],
                             rhs=identf, is_transpose=True)
            nc.scalar.copy(uT_all[:, idx, :], pu[0:16, 0:128])

    # ---- iteration 0: packed matmuls, all capsules, both halves ----
    pump(WRING)
    for lo, hi, st0name in ((0, 8, "sT0A"), (8, 10, "sT0B")):
        ncaps = hi - lo
        pv0 = psv.tile([128, 512], F32, tag="pv")
        for j in range(KT):
            i, rb = j // RB, j % RB
            nc.tensor.matmul(
                pv0[0:16 * ncaps, 0:256],
                lhsT=wall[:, i, rb, lo:hi, :].rearrange(
                    "p c o -> p (c o)"),
                rhs=xT[:, i, rb, :, :].rearrange("p h b -> p (h b)"),
                start=(j == 0), stop=(j == KT - 1))
        sT0 = big.tile([16 * ncaps, 256], F32, tag=st0name)
        nc.scalar.mul(sT0, pv0[0:16 * ncaps, 0:256], 1.0 / R)
        for m in range(ncaps // 2):
            for h in range(2):
                src = sT0[32 * m:32 * m + 32, 128 * h:128 * h + 128]
                ps2 = pst.tile([128, 512], F32, tag="ptr")
                nc.tensor.matmul(ps2[:, 0:32], lhsT=src,
                                 rhs=identf[32 * m:32 * m + 32,
                                            32 * m:32 * m + 32],
                                 is_transpose=True,
                                 tile_position=(32 * m, 0))
                items = [(2 * (lo + 2 * m + n) + h,
                          sv_copy(ps2[:, 16 * n:16 * n + 16]))
                         for n in range(2)]
                squash_batch(0, items)

    def stage_A(it, c):
        """V-matmuls + drains + mult + tree for both halves of c -> l."""
        t1s = [t1_pool.tile([128, R, 4], F16, tag=f"t1_{h}", name=f"t1_{h}")
               for h in range(2)]
        for j in range(NCH):
            wt = chunk_tiles.pop((it, c, j))
            for h in range(2):
                idx = 2 * c + h
                twb = twb_pool.tile([128, 128, I], F16, tag="twb")
                pv = psv.tile([128, 1024], F32, tag="pv")
                for half in range(2):
                    nc.tensor.matmul(
                        pv[:, 512 * half:512 * half + 512],
                        lhsT=uT_all[:, idx, :],
                        rhs=wt[:, 512 * half:512 * half + 512],
                        start=True, stop=True)
                xslice = xus[h][:, 128 * j:128 * j + 128, :]
                if j >= NCH - JDVE:
                    # DVE multiplies straight from PSUM (f32, 1x)
                    nc.vector.tensor_mul(
                        twb, xslice, pv.rearrange("p (r i) -> p r i", i=I))
                else:
                    v16 = v16_pool.tile([128, 1024], F16, tag="v16")
                    nc.scalar.copy(v16, pv)
                    mul_eng = nc.gpsimd if j < JPOOL else nc.vector
                    mul_eng.tensor_mul(
                        twb, xslice, v16.rearrange("p (r i) -> p r i", i=I))
                nc.vector.tensor_add(t1s[h][:, 128 * j:128 * j + 128, :],
                                     twb[:, :, 0:4], twb[:, :, 4:8])
            pump(1)
        ls = []
        for h in range(2):
            t1 = t1s[h]
            nc.vector.tensor_add(t1[:, :, 0:2], t1[:, :, 0:2], t1[:, :, 2:4])
            l_t = l_pool.tile([128, R], F32, tag="l")
            nc.vector.tensor_add(l_t, t1[:, :, 0], t1[:, :, 1])
            ls.append((l_t, t1))
        return ls

    def stage_B1_exp(ls):
        """exp with accum, then Copy-with-scale normalization -> e (f16)."""
        es = []
        for l_t, t1 in ls:
            d = small.tile([128, 1], F32, tag="d")
            escr = t1.bitcast(F32)[:, :, 0]
            nc.scalar.activation(escr, l_t, AF.Exp, accum_out=d)
            rd = small.tile([128, 1], F32, tag="rd")
            nc.vector.reciprocal(rd, d)
            e_t = e_pool.tile([128, R], F16, tag="e")
            nc.scalar.activation(e_t, escr, AF.Copy, scale=rd)
            es.append(e_t)
        return es

    def stage_B1_transp(es):
        """PE transposes of e -> eT (f16)."""
        eTs = []
        for h in range(2):
            eT = eT_pool.tile([128, RB, 128], F16, tag="eT")
            eTf = eT.rearrange("p rb b -> p (rb b)")
            for g, cnt in ((0, 4), (4, 4), (8, 1)):
                pseT = pse.tile([128, 512], F32, tag="pteT",
                                name="pseT").bitcast(F16)
                for sub in range(cnt):
                    rb = g + sub
                    nc.tensor.matmul(
                        pseT[:, 128 * sub:128 * sub + 128],
                        lhsT=es[h][:, 128 * rb:128 * rb + 128],
                        rhs=ident16, is_transpose=True)
                if ESPLIT and g % 2 == 1:
                    nc.vector.tensor_copy(eTf[:, 128 * g:128 * (g + cnt)],
                                          pseT[:, 0:128 * cnt])
                else:
                    nc.scalar.copy(eTf[:, 128 * g:128 * (g + cnt)],
                                   pseT[:, 0:128 * cnt])
            eTs.append(eT)
        return eTs

    def y_mult(eTs, y2, qq):
        for h in range(2):
            eT = eTs[h]
            e_bcast = bass.AP(
                tensor=eT.tensor, offset=eT.offset,
                ap=[eT.ap[0], [0, 2], [128, RB], [1, 128]],
            )
            eng = nc.gpsimd if (2 * qq + h) < YPOOL else nc.vector
            eng.tensor_mul(y2[:, :, :, h, :],
                           xT[:, 2 * qq:2 * qq + 2, :, h, :], e_bcast)

    def stage_B2_y(state):
        """first half of the y products (DVE/Pool) — emitted early."""
        eTs = state["eTs"]
        ys = []
        for qq in range(2):
            y2 = y_pool.tile([128, 2, RB, 2, BH], F16, tag="y2")
            y_mult(eTs, y2, qq)
            ys.append(y2)
        state["ys"] = ys

    def stage_B2_s(it, c, state):
        """remaining y interleaved with the s-matmul chain, then squash."""
        eTs = state["eTs"]
        ys = state["ys"]
        ps_s = pss.tile([16, 2, BH], F32, tag="ps_s")
        for qq in range(4):
            if qq >= 2:
                y2 = y_pool.tile([128, 2, RB, 2, BH], F16, tag="y2")
                y_mult(eTs, y2, qq)
                ys.append(y2)
            for jj in range(2 * RB):
                ii, rb = 2 * qq + jj // RB, jj % RB
                j = qq * 2 * RB + jj
                nc.tensor.matmul(
                    ps_s, lhsT=wall[:, ii, rb, c, :],
                    rhs=ys[qq][:, jj // RB, rb, :, :].rearrange(
                        "p h b -> p (h b)"),
                    start=(j == 0), stop=(j == KT - 1))
        sTp = sT_pool.tile([16, 2, BH], F32, tag="sTp")
        nc.scalar.copy(sTp, ps_s)
        items = []
        for h in range(2):
            ps2 = pst.tile([128, 512], F32, tag="ptr")
            nc.tensor.matmul(ps2[:, 0:16], lhsT=sTp[:, h, :],
                             rhs=identf[0:16, 0:16], is_transpose=True)
            items.append((2 * c + h, sv_copy(ps2[:, 0:16])))
        squash_batch(it, items)

    # --- 3-deep software pipeline over the 20 (it, c) pairs ---
    pairs = [(it, c) for it in (1, 2) for c in range(C)]
    st = {}  # pair index -> state dict
    n_pairs = len(pairs)
    for n in range(n_pairs + 2):
        if 0 <= n - 2 < n_pairs:
            s2 = st[n - 2]
            s2["eTs"] = stage_B1_transp(s2.pop("es"))
            stage_B2_y(s2)
        if 0 <= n - 1 < n_pairs and "ls" in st.get(n - 1, {}):
            s1 = st[n - 1]
            s1["es"] = stage_B1_exp(s1.pop("ls"))
        if n < n_pairs:
            st[n] = {"ls": stage_A(*pairs[n])}
        if 0 <= n - 2 < n_pairs:
            it2, c2 = pairs[n - 2]
            stage_B2_s(it2, c2, st.pop(n - 2))


def _prefer_ln_exp_table(bacc):
    """Steer the ACT-table chooser to set 6 (natural_log_exp_and_others),
    which covers every func this kernel uses (exp/ln/square/copy/identity),
    so the schedule needs a single table load instead of flipping between
    the exp-only and ln-only sets every iteration. Positions (and thus
    act_func_set_ids) of all sets are preserved; earlier sets are just made
    unselectable."""
    if getattr(bacc, "_caps_act_tables_patched", False):
        return
    orig = bacc.get_activation_tables

    def patched(arch):
        tables = orig(arch)
        out = {}
        for i, (name, funcs) in enumerate(tables.items()):
            out[name] = funcs if i >= 6 else set()
        return out

    bacc.get_activation_tables = patched
    bacc._caps_act_tables_patched = True


def build_program():
    from concourse import bacc
    _prefer_ln_exp_table(bacc)
    nc = bacc.Bacc("TRN2", target_bir_lowering=False, debug=False,
                   num_devices=NCORES)
    xu_ap = nc.declare_dram_parameter("xu", [2, 128, R, I], F16,
                                      isOutput=False).ap()
    xT_ap = nc.declare_dram_parameter("xT", [128, I, RB, 2, BH], F16,
                                      isOutput=False).ap()
    wall_ap = nc.declare_dram_parameter("wall", [128, I, RB, C, O], F16,
                                        isOutput=False).ap()
    wT_ap = nc.declare_dram_parameter("wT", [C, O, K], F32R,
                                      isOutput=False).ap()
    out = nc.declare_dram_parameter("out", [C, 2, BH, O], F32,
                                    isOutput=True).ap()
    with tile.TileContext(nc) as tc:
        _caps_kernel(tc, out, xu_ap, xT_ap, wall_ap, wT_ap)
    nc.compile()
    return nc


def make_in_maps(x, w):
    x = np.ascontiguousarray(x, dtype=np.float32)
    w = np.ascontiguousarray(w, dtype=np.float32)
    xu = x.reshape(2, 128, R, I).astype(np.float16)
    # xT[p, i, rb, h, b] = x[h*128+b, rb*128+p, i]
    xT = np.ascontiguousarray(
        x.reshape(2, 128, RB, 128, I).transpose(3, 4, 2, 0, 1)
    ).astype(np.float16)
    # wall[p, i, rb, c, o] = w[c, rb*128+p, i, o]
    wall = np.ascontiguousarray(
        w.reshape(C, RB, 128, I, O).transpose(2, 3, 1, 0, 4)
    ).astype(np.float16)
    # wT[c, o, k] = w[c, k // I, k % I, o]
    wT = np.ascontiguousarray(w.reshape(C, K, O).transpose(0, 2, 1))
    return [{"xu": xu, "xT": xT, "wall": wall, "wT": wT}]


def kernel(x: np.ndarray, route_weights: np.ndarray) -> np.ndarray:
    from concourse.bass_utils import run_bass_kernel_spmd

    in_maps = make_in_maps(x, route_weights)
    nc = build_program()
    res = run_bass_kernel_spmd(nc, in_maps, list(range(NCORES)))
    o = res.results[0]["out"]  # [C, 2, BH, O]
    return np.ascontiguousarray(o.reshape(C, B, 1, O), dtype=np.float32)


if __name__ == "__main__":
    rng = np.random.default_rng(0)
    x = rng.normal(size=(B, R, I)).astype(np.float32)
    w = rng.normal(size=(C, R, I, O)).astype(np.float32)
    out = kernel(x=x, route_weights=w)
    print(out.shape, out.dtype, np.abs(out).mean())
